# revision 39
# baseline (speedup 1.0000x reference)
"""CTPN loss kernel for 8 Trainium2 NeuronCores.

v3 strategy (dense smooth-L1 pass; tiny gather only for dup cells + cls):
  * The H*W=24576 spatial positions are split into 8 contiguous slices of
    3072; core c holds the dense map data for its slice as a bf16
    [128, 1536] tile of "channel-half" rows (score rows pair-interleaved
    by quarter, as the cls gather wants them).  Within each 16-partition
    gather group, vp/sd rows sit at slots {0..3, 8..11} and score rows at
    {4..7, 12..15}, so the target tile T (needed only for vp/sd rows) can
    be DMA'd with a partition-strided pattern that loads every DMA engine
    equally.
  * T holds the per-cell regression targets scattered into the data
    layout, defaulting to a copy of the data so non-anchor cells subtract
    to exactly 0.  Smooth-L1 uses
        sl1(d) = 0.5*d^2 - 0.5*(max(|d|,1)-1)^2
    with Sum(d^2) from a DVE tensor_tensor_reduce and Sum((z-1)^2) from
    one ACT Square (bias=-1) with free accumulation.  z=max(|d|,1) comes
    from bf16 bit tricks (clear sign bit; unsigned max vs 0x3F80).
  * Cells referenced by MORE than one regression entry (random index
    collisions) can hold only one dense target; the 2nd+ entries go
    through a small InstIndirectCopy gather (a few dozen columns), along
    with the 2*128 classification logits (pair-adjacent columns,
    ce = Softplus(first - second), single-table activation).
  * Per-partition partial sums go back to the host, which applies the
    per-segment divisors (1/(2*Nv), 1/No, 1/Ns) and sums across cores
    (the all-reduce of the sharding hint).  Score-row partitions of the
    dense sums are garbage (their T rows are never shipped) and are
    simply not read by the host.
"""

import sys

sys.path.insert(0, "/opt/trn_rl_repo")

import numpy as np

import concourse.bacc as bacc
import concourse.tile as tile
from concourse import mybir
from concourse import bass_utils

# ---------------- problem constants (hardcoded per contract) ----------------
H, W, K = 128, 192, 10
HW = H * W                     # 24576
N_CORES = 8
PPC = HW // N_CORES            # 3072 positions per core
COLS = 1536                    # slot width (elements) = half of PPC
QCOLS = 768                    # quarter width (score slots are pair-interleaved)
NS = 128.0
NV_REG = 20000
NO_REG = 5000

# ---- static unit tables ----------------------------------------------------
# unit kinds: 'vp' (a, h) -> 2 partitions; 'sd' (a, h) -> 1; 'sc' (a, q) -> 1
UNITS = []
for a in range(K):
    for h in range(2):
        UNITS.append(("vp", a, h))
for a in range(K):
    for h in range(2):
        UNITS.append(("sd", a, h))
for a in range(K):
    for q in range(4):
        UNITS.append(("sc", a, q))
N_UNITS = len(UNITS)  # 80
UNIT_NPART = {"vp": 2, "sd": 1, "sc": 1}
REG_UNITS = [ui for ui, u in enumerate(UNITS) if u[0] != "sc"]
SC_UNITS = [ui for ui, u in enumerate(UNITS) if u[0] == "sc"]

_cache = {}


def _bf16(x):
    """f32 ndarray -> uint16 bf16 bits (round to nearest even)."""
    u = np.ascontiguousarray(np.asarray(x, np.float32)).view(np.uint32)
    return (((u + 0x7FFF) + ((u >> 16) & 1)) >> 16).astype(np.uint16)


def _pack_units(main_cnt, cls_cnt):
    """Assign units to 16-partition groups and partition slots.

    vp/sd units go to group-relative slots {0..3, 8..11} (two runs of 4;
    vp needs 2 adjacent slots in one run), sc units to {4..7, 12..15}.
    Balances main_cnt over groups for vp/sd and cls_cnt for sc.
    Returns upart[N_UNITS], ugroup[N_UNITS], per-group (gmain, gcls).
    """
    upart = [-1] * N_UNITS
    ugroup = [-1] * N_UNITS
    gmain = [0] * 8
    gcls = [0] * 8
    # run free-lists per group: [run0_next, run1_next] relative next slot
    reg_runs = [[0, 0] for _ in range(8)]   # used counts in runs of 4
    # vp first (need adjacency), heaviest first
    vps = sorted((ui for ui in REG_UNITS if UNITS[ui][0] == "vp"),
                 key=lambda ui: -main_cnt[ui])
    sds = sorted((ui for ui in REG_UNITS if UNITS[ui][0] == "sd"),
                 key=lambda ui: -main_cnt[ui])
    for ui in vps:
        best, bestv = -1, None
        for g in range(8):
            if max(r for r in (4 - reg_runs[g][0], 4 - reg_runs[g][1])) < 2:
                continue
            if bestv is None or gmain[g] < bestv:
                best, bestv = g, gmain[g]
        assert best >= 0, "vp packing overflow"
        g = best
        r = 0 if 4 - reg_runs[g][0] >= 2 else 1
        base = 16 * g + (0 if r == 0 else 8) + reg_runs[g][r]
        reg_runs[g][r] += 2
        upart[ui] = base
        ugroup[ui] = g
        gmain[g] += int(main_cnt[ui])
    for ui in sds:
        best, bestv = -1, None
        for g in range(8):
            if reg_runs[g][0] >= 4 and reg_runs[g][1] >= 4:
                continue
            if bestv is None or gmain[g] < bestv:
                best, bestv = g, gmain[g]
        assert best >= 0, "sd packing overflow"
        g = best
        r = 0 if reg_runs[g][0] < 4 else 1
        base = 16 * g + (0 if r == 0 else 8) + reg_runs[g][r]
        reg_runs[g][r] += 1
        upart[ui] = base
        ugroup[ui] = g
        gmain[g] += int(main_cnt[ui])
    # sc units into slots {4..7, 12..15}
    sc_runs = [[0, 0] for _ in range(8)]
    scs = sorted(SC_UNITS, key=lambda ui: -cls_cnt[ui])
    for ui in scs:
        best, bestv = -1, None
        for g in range(8):
            if sc_runs[g][0] >= 4 and sc_runs[g][1] >= 4:
                continue
            if bestv is None or gcls[g] < bestv:
                best, bestv = g, gcls[g]
        assert best >= 0, "sc packing overflow"
        g = best
        r = 0 if sc_runs[g][0] < 4 else 1
        base = 16 * g + (4 if r == 0 else 12) + sc_runs[g][r]
        sc_runs[g][r] += 1
        upart[ui] = base
        ugroup[ui] = g
        gcls[g] += int(cls_cnt[ui])
    return upart, ugroup, gmain, gcls


def _first_mask(cids):
    """Boolean mask selecting one 'first' entry per distinct cell id."""
    order = np.argsort(cids, kind="stable")
    sc = cids[order]
    fs = np.ones(len(cids), np.bool_)
    if len(cids):
        fs[1:] = sc[1:] != sc[:-1]
    mask = np.zeros(len(cids), np.bool_)
    mask[order] = fs
    return mask


# ln(1+y) on [0,1], degree-5 power-basis fit (max abs err ~1e-5)
_SPC = (9.975032552178575e-06, 0.9992354838332733, -0.4902307234234048,
        0.28527268109057763, -0.13158182508877841, 0.030449004538680213)


def _build_bass(NVS, C0S, WB, NCLS):
    nc = bacc.Bacc("TRN2", target_bir_lowering=False)
    NIS = NVS // 16
    MEGA = nc.dram_tensor("mega", [128, WB], mybir.dt.uint8, kind="ExternalInput")
    OUT = nc.dram_tensor("out", [128, 12], mybir.dt.float32,
                         kind="ExternalOutput")

    o_gs = 3072                     # gather dst; NOT covered by any DMA
    o_idx = o_gs + 2 * NVS          # (avoids a WAW fence before the gather)
    o_mc = o_idx + 2 * NIS
    o_T = ((o_mc + NCLS + 3) // 4) * 4
    o_tms = o_T + 3072              # small targets right after dense T

    f32 = mybir.dt.float32
    bf16 = mybir.dt.bfloat16
    u16 = mybir.dt.uint16
    u32 = mybir.dt.uint32
    AL = mybir.AluOpType
    ACT = mybir.ActivationFunctionType
    with tile.TileContext(nc) as tc:
        with tc.tile_pool(name="p", bufs=1) as pool:
            mega = pool.tile([128, WB], mybir.dt.uint8)
            # DMA issue order = consumption order: chunk 0's inputs (data0,
            # T0) first so the scalar engine starts ASAP; then data1 + idx
            # for the gather; then chunk 1's targets.  The gather dst (gs)
            # is covered by NO DMA, avoiding a WAW fence before the gather.
            nc.sync.dma_start(mega[:, o_idx:o_T], MEGA[:, o_idx:o_T])
            nc.scalar.dma_start(mega[:, o_T:o_T + 1536],
                                MEGA[:, o_T:o_T + 1536])
            nc.sync.dma_start(mega[:, 0:1536], MEGA[:, 0:1536])
            nc.scalar.dma_start(mega[:, o_T + 1536:WB],
                                MEGA[:, o_T + 1536:WB])
            nc.sync.dma_start(mega[:, 1536:3072], MEGA[:, 1536:3072])

            # warm activations: Exp and Square share one act table; no other
            # activation functions are used anywhere -> no mid-kernel loads
            warm = pool.tile([128, 4], f32)
            nc.scalar.activation(warm[:, 0:2], warm[:, 2:4], ACT.Exp)
            nc.scalar.activation(warm[:, 0:2], warm[:, 2:4], ACT.Square)

            data_v = mega[:, 0:3072].bitcast(bf16)               # [128,1536]
            gs_v = mega[:, o_gs:o_gs + 2 * NVS].bitcast(bf16)    # [128,NVS]
            idx_v = mega[:, o_idx:o_idx + 2 * NIS].bitcast(u16)
            mc_v = mega[:, o_mc:o_mc + NCLS]                     # u8

            P = pool.tile([128, 12], f32)

            # ---- small gather: dup-cell entries + cls logit pairs, landing
            # directly behind the dense data so dense chunk 1 covers them --
            nc.gpsimd.indirect_copy(
                gs_v, data_v, idx_v, i_know_ap_gather_is_preferred=True
            )

            # smooth-l1 partial sums for one [128, w] bf16 diff tile D via
            #   sl1(d) = 0.5*d^2 - 0.5*r^2,  r = relu(|d| - 1)
            # DVE computes r (sign-clear then clamp); ACT Square with free
            # accumulation produces both sums on the scalar engine.
            def sl1_vops(D, w):
                Ab = pool.tile([128, w], bf16)
                nc.vector.tensor_scalar(Ab[:].bitcast(u16), D[:].bitcast(u16),
                                        0x7FFF, None, AL.bitwise_and)
                R = pool.tile([128, w], bf16)
                nc.vector.tensor_scalar(R[:], Ab[:], 1.0, 0.0,
                                        AL.subtract, AL.max)
                return R

            def sl1_sop(D, R, w, pc):
                SQ = pool.tile([128, w], bf16)
                nc.scalar.activation(SQ[:], D[:], ACT.Square,
                                     accum_out=P[:, pc:pc + 1])
                SR = pool.tile([128, w], bf16)
                nc.scalar.activation(SR[:], R[:], ACT.Square,
                                     accum_out=P[:, pc + 1:pc + 2])

            # ---- dense chunk 0 (cols 0:768) ------------------------------
            D0 = pool.tile([128, 768], bf16)
            nc.vector.tensor_tensor(D0[:], data_v[:, 0:768],
                                    mega[:, o_T:o_T + 1536].bitcast(bf16),
                                    op=AL.subtract)
            R0 = sl1_vops(D0, 768)

            # ---- cls prep (gather-dependent, tiny) -----------------------
            dc = pool.tile([128, NCLS], f32)
            nc.vector.tensor_tensor(dc[:], gs_v[:, C0S:NVS:2],
                                    gs_v[:, C0S + 1:NVS:2], op=AL.subtract)
            adc = pool.tile([128, NCLS], f32)
            nc.vector.tensor_scalar(adc[:].bitcast(u32), dc[:].bitcast(u32),
                                    0x7FFFFFFF, None, AL.bitwise_and)

            # ---- dense chunk 1 (cols 768:1536 + the gathered columns) ----
            W1 = 768 + NVS
            D1 = pool.tile([128, W1], bf16)
            nc.vector.tensor_tensor(
                D1[:], mega[:, 1536:o_gs + 2 * NVS].bitcast(bf16),
                mega[:, o_T + 1536:o_tms + 2 * NVS].bitcast(bf16),
                op=AL.subtract)
            R1 = sl1_vops(D1, W1)

            # ---- scalar engine stream ------------------------------------
            sl1_sop(D0, R0, 768, 0)
            # y = exp(-|dc|), emitted before chunk 1's squares so the DVE
            # cls-poly tail overlaps them
            y = pool.tile([128, NCLS], f32)
            nc.scalar.activation(y[:], adc[:], ACT.Exp, scale=-1.0)
            sl1_sop(D1, R1, W1, 2)

            # ---- cls tail: ce = relu(dc) + ln1p(y) via degree-3 poly -----
            c0, c1, c2, c3 = (0.0005027216331514595, 0.9823971197982746,
                              -0.3971182964499659, 0.10774685617806001)
            y2 = pool.tile([128, NCLS], f32)
            nc.vector.tensor_tensor(y2[:], y[:], y[:], op=AL.mult)
            q0 = pool.tile([128, NCLS], f32)    # c0 + c1 y
            nc.vector.tensor_scalar(q0[:], y[:], c1, c0, AL.mult, AL.add)
            q1 = pool.tile([128, NCLS], f32)    # c2 + c3 y
            nc.vector.tensor_scalar(q1[:], y[:], c3, c2, AL.mult, AL.add)
            t1 = pool.tile([128, NCLS], f32)
            nc.vector.tensor_tensor(t1[:], y2[:], q1[:], op=AL.mult)
            r = pool.tile([128, NCLS], f32)
            nc.vector.tensor_scalar(r[:], dc[:], 0.0, None, AL.max)
            s1 = pool.tile([128, NCLS], f32)
            nc.vector.tensor_tensor(s1[:], q0[:], t1[:], op=AL.add)
            ce = pool.tile([128, NCLS], f32)
            nc.vector.tensor_tensor(ce[:], s1[:], r[:], op=AL.add)
            cj = pool.tile([128, NCLS], f32)
            nc.vector.tensor_tensor(cj[:], ce[:], mc_v, op=AL.mult)
            nc.vector.tensor_reduce(P[:, 4:5], cj[:],
                                    axis=mybir.AxisListType.X, op=AL.add)
            nc.vector.memset(P[:, 5:12], 0.0)

            nc.sync.dma_start(OUT[:, :], P[:])
    nc.compile()
    return nc


def kernel(**inputs):
    score = np.asarray(inputs["score"], dtype=np.float32)[0]            # [20,H,W]
    vp = np.asarray(inputs["vertical_pred"], dtype=np.float32)[0]
    side = np.asarray(inputs["side_refinement"], dtype=np.float32)[0]   # [10,H,W]
    pidx = np.asarray(inputs["positive"])
    nidx = np.asarray(inputs["negative"])
    vidx = np.asarray(inputs["vertical_reg_idx"])
    vtgt = np.asarray(inputs["vertical_reg_tgt"], dtype=np.float32)
    sidx = np.asarray(inputs["side_reg_idx"])
    stgt = np.asarray(inputs["side_reg_tgt"], dtype=np.float32)

    score_bf = _bf16(score.reshape(2 * K, HW))
    vp_bf = _bf16(vp.reshape(2 * K, HW))
    side_bf = _bf16(side.reshape(K, HW))
    vtgt_bf = _bf16(vtgt)      # [Nv,2] u16
    stgt_bf = _bf16(stgt)      # [No]   u16

    def fields(idx):
        x = idx[:, 0].astype(np.int64)
        y = idx[:, 1].astype(np.int64)
        a = idx[:, 2].astype(np.int64)
        pos = y * W + x
        return a, pos // PPC, pos % PPC

    va, vcore, vposl = fields(vidx)
    sa, score_, sposl = fields(sidx)
    pa, pcore, pposl = fields(pidx)
    na, ncore, nposl = fields(nidx)

    # --- per-entry unit / in-row-offset ----------------------------------
    v_h = vposl // COLS
    v_u = (vposl % COLS).astype(np.int64)
    v_unit = (va * 2 + v_h).astype(np.int64)                 # vp units 0..19
    s_h = sposl // COLS
    s_u = (sposl % COLS).astype(np.int64)
    s_unit = (20 + sa * 2 + s_h).astype(np.int64)            # sd units 20..39
    p_q = pposl // QCOLS
    p_u = (2 * (pposl % QCOLS)).astype(np.int64)
    p_unit = (40 + pa * 4 + p_q).astype(np.int64)            # sc units 40..79
    n_q = nposl // QCOLS
    n_u = (2 * (nposl % QCOLS)).astype(np.int64)
    n_unit = (40 + na * 4 + n_q).astype(np.int64)

    # --- dup split: one dense anchor per distinct cell --------------------
    v_cid = (vcore * K + va) * PPC + vposl                   # vp pair-cell id
    s_cid = K * HW + (score_ * K + sa) * PPC + sposl         # sd cell id
    v_first = _first_mask(v_cid)
    s_first = _first_mask(s_cid)

    main_core = np.concatenate([vcore, score_])
    main_unit = np.concatenate([v_unit, s_unit])
    main_u = np.concatenate([v_u, s_u])
    main_t0 = np.concatenate([vtgt_bf[:, 0], stgt_bf])       # bf16 bits
    main_t1 = np.concatenate([vtgt_bf[:, 1],
                              np.zeros_like(stgt_bf)])
    main_isv = np.concatenate(
        [np.ones(len(vidx), np.bool_), np.zeros(len(sidx), np.bool_)])
    main_first = np.concatenate([v_first, s_first])

    cls_core = np.concatenate([pcore, ncore])
    cls_unit = np.concatenate([p_unit, n_unit])
    cls_u = np.concatenate([p_u, n_u])
    cls_ispos = np.concatenate(
        [np.ones(len(pidx), np.bool_), np.zeros(len(nidx), np.bool_)])

    dup_cnt = np.zeros((N_CORES, N_UNITS), np.int64)
    dsel_all = ~main_first
    np.add.at(dup_cnt, (main_core[dsel_all], main_unit[dsel_all]), 1)
    cls_cnt = np.zeros((N_CORES, N_UNITS), np.int64)
    np.add.at(cls_cnt, (cls_core, cls_unit), 2)

    packs = [_pack_units(dup_cnt[c], cls_cnt[c]) for c in range(N_CORES)]
    c0s = max(max(p[2]) for p in packs)
    c0s += c0s % 2
    max_cls = max(max(p[3]) for p in packs)
    NVS = c0s + max_cls
    NVS = max(16, ((NVS + 15) // 16) * 16)
    NCLS = (NVS - c0s) // 2
    NIS = NVS // 16
    o_gs = 3072
    o_idx = o_gs + 2 * NVS
    o_mc = o_idx + 2 * NIS
    o_T = ((o_mc + NCLS + 3) // 4) * 4
    o_tms = o_T + 3072
    WB = ((o_tms + 2 * NVS + 3) // 4) * 4

    key = (NVS, c0s)
    if key not in _cache:
        _cache[key] = _build_bass(NVS, c0s, WB, NCLS)
    nc = _cache[key]

    in_maps = []
    wvec_v = np.zeros((N_CORES, 128), np.float32)
    wvec_o = np.zeros((N_CORES, 128), np.float32)
    for c in range(N_CORES):
        upart, ugroup, gmain, gcls = packs[c]
        uparta = np.asarray(upart, np.int64)

        # dense data tile (bf16 bits) --------------------------------------
        df = np.zeros((128, COLS), np.uint16)
        base = c * PPC
        for ui, (kind, a, hq) in enumerate(UNITS):
            p0 = upart[ui]
            if kind == "vp":
                sl = slice(base + hq * COLS, base + (hq + 1) * COLS)
                df[p0] = vp_bf[2 * a, sl]
                df[p0 + 1] = vp_bf[2 * a + 1, sl]
                wvec_v[c, p0] = wvec_v[c, p0 + 1] = 1.0 / (2.0 * NV_REG)
            elif kind == "sd":
                sl = slice(base + hq * COLS, base + (hq + 1) * COLS)
                df[p0] = side_bf[a, sl]
                wvec_o[c, p0] = 1.0 / NO_REG
            else:  # sc, pair-interleaved quarter
                sl = slice(base + hq * QCOLS, base + (hq + 1) * QCOLS)
                df[p0, 0::2] = score_bf[2 * a, sl]
                df[p0, 1::2] = score_bf[2 * a + 1, sl]

        # dense target tile: data copy, then first-occurrence targets ------
        Tf = df.copy()
        msel = (main_core == c) & main_first
        Tf[uparta[main_unit[msel]], main_u[msel]] = main_t0[msel]
        vsel = msel & main_isv
        Tf[uparta[main_unit[vsel]] + 1, main_u[vsel]] = main_t1[vsel]

        # small gather: dup entries + cls pairs ----------------------------
        idxs = np.zeros((128, NIS), np.uint16)
        ucol = np.zeros((8, NVS), np.int64)
        mcls = np.zeros((128, NCLS), np.uint8)

        gq_main = [0] * 8
        gq_cls = [0] * 8

        def put_idx(g, col, val):
            idxs[16 * g + col % 16, col // 16] = val
            ucol[g, col] = val

        ov_p, ov_c, ov_t = [], [], []
        dsel = (main_core == c) & ~main_first
        for u, ui, t0, t1, isv in zip(main_u[dsel], main_unit[dsel],
                                      main_t0[dsel], main_t1[dsel],
                                      main_isv[dsel]):
            g = ugroup[ui]
            col = gq_main[g]
            gq_main[g] += 1
            put_idx(g, col, u)
            p0 = upart[ui]
            ov_p.append(p0); ov_c.append(col); ov_t.append(t0)
            if isv:
                ov_p.append(p0 + 1); ov_c.append(col); ov_t.append(t1)

        csel = cls_core == c
        for u, ui, ispos in zip(cls_u[csel], cls_unit[csel],
                                cls_ispos[csel]):
            g = ugroup[ui]
            i = gq_cls[g]
            gq_cls[g] += 1
            colf = c0s + 2 * i
            if ispos:
                put_idx(g, colf, u)
                put_idx(g, colf + 1, u + 1)
            else:
                put_idx(g, colf, u + 1)
                put_idx(g, colf + 1, u)
            mcls[upart[ui], i] = 1

        # small TM (single plane, bf16): default = the gathered bits, so
        # junk columns (incl. the whole cls region) subtract to exactly 0
        tms = np.empty((128, NVS), np.uint16)
        for g in range(8):
            tms[16 * g:16 * g + 16] = df[16 * g:16 * g + 16][:, ucol[g]]
        if ov_p:
            tms[np.array(ov_p), np.array(ov_c)] = np.array(ov_t, np.uint16)

        mega = np.zeros((128, WB), np.uint8)
        mega[:, 0:3072] = df.view(np.uint8)
        mega[:, o_idx:o_idx + 2 * NIS] = idxs.view(np.uint8)
        mega[:, o_mc:o_mc + NCLS] = mcls
        mega[:, o_T:o_T + 3072] = Tf.view(np.uint8)
        mega[:, o_tms:o_tms + 2 * NVS] = tms.view(np.uint8)
        in_maps.append({"mega": mega})

    res = bass_utils.run_bass_kernel_spmd(
        nc, in_maps, core_ids=list(range(N_CORES)))

    v_loss = np.float32(0.0)
    o_loss = np.float32(0.0)
    cls_sum = np.float32(0.0)
    for c in range(N_CORES):
        P = res.results[c]["out"]      # [128, 12]
        # per-partition sl1 sum: 0.5*(Sum(d^2) - Sum(r^2)), chunks 0+1
        S = 0.5 * ((P[:, 0] + P[:, 2]) - (P[:, 1] + P[:, 3]))
        m = (wvec_v[c] != 0) | (wvec_o[c] != 0)
        S = np.where(m, S, np.float32(0))
        v_loss += np.float32(np.dot(S, wvec_v[c]))
        o_loss += np.float32(np.dot(S, wvec_o[c]))
        cls_sum += np.float32(P[:, 4].sum())
    cls_loss = np.float32(cls_sum / NS)
    loss = np.float32(cls_loss + v_loss + o_loss)
    return (np.float32(loss), np.float32(cls_loss), np.float32(v_loss),
            np.float32(o_loss))


# revision 40
# speedup vs baseline: 1.0186x; 1.0186x over previous
"""CTPN loss kernel for 8 Trainium2 NeuronCores.

v3 strategy (dense smooth-L1 pass; tiny gather only for dup cells + cls):
  * The H*W=24576 spatial positions are split into 8 contiguous slices of
    3072; core c holds the dense map data for its slice as a bf16
    [128, 1536] tile of "channel-half" rows (score rows pair-interleaved
    by quarter, as the cls gather wants them).  Within each 16-partition
    gather group, vp/sd rows sit at slots {0..3, 8..11} and score rows at
    {4..7, 12..15}, so the target tile T (needed only for vp/sd rows) can
    be DMA'd with a partition-strided pattern that loads every DMA engine
    equally.
  * T holds the per-cell regression targets scattered into the data
    layout, defaulting to a copy of the data so non-anchor cells subtract
    to exactly 0.  Smooth-L1 uses
        sl1(d) = 0.5*d^2 - 0.5*(max(|d|,1)-1)^2
    with Sum(d^2) from a DVE tensor_tensor_reduce and Sum((z-1)^2) from
    one ACT Square (bias=-1) with free accumulation.  z=max(|d|,1) comes
    from bf16 bit tricks (clear sign bit; unsigned max vs 0x3F80).
  * Cells referenced by MORE than one regression entry (random index
    collisions) can hold only one dense target; the 2nd+ entries go
    through a small InstIndirectCopy gather (a few dozen columns), along
    with the 2*128 classification logits (pair-adjacent columns,
    ce = Softplus(first - second), single-table activation).
  * Per-partition partial sums go back to the host, which applies the
    per-segment divisors (1/(2*Nv), 1/No, 1/Ns) and sums across cores
    (the all-reduce of the sharding hint).  Score-row partitions of the
    dense sums are garbage (their T rows are never shipped) and are
    simply not read by the host.
"""

import sys

sys.path.insert(0, "/opt/trn_rl_repo")

import numpy as np

import concourse.bacc as bacc
import concourse.tile as tile
from concourse import mybir
from concourse import bass_utils

# ---------------- problem constants (hardcoded per contract) ----------------
H, W, K = 128, 192, 10
HW = H * W                     # 24576
N_CORES = 8
PPC = HW // N_CORES            # 3072 positions per core
COLS = 1536                    # slot width (elements) = half of PPC
QCOLS = 768                    # quarter width (score slots are pair-interleaved)
NS = 128.0
NV_REG = 20000
NO_REG = 5000

# ---- static unit tables ----------------------------------------------------
# unit kinds: 'vp' (a, h) -> 2 partitions; 'sd' (a, h) -> 1; 'sc' (a, q) -> 1
UNITS = []
for a in range(K):
    for h in range(2):
        UNITS.append(("vp", a, h))
for a in range(K):
    for h in range(2):
        UNITS.append(("sd", a, h))
for a in range(K):
    for q in range(4):
        UNITS.append(("sc", a, q))
N_UNITS = len(UNITS)  # 80
UNIT_NPART = {"vp": 2, "sd": 1, "sc": 1}
REG_UNITS = [ui for ui, u in enumerate(UNITS) if u[0] != "sc"]
SC_UNITS = [ui for ui, u in enumerate(UNITS) if u[0] == "sc"]

_cache = {}


def _bf16(x):
    """f32 ndarray -> uint16 bf16 bits (round to nearest even)."""
    u = np.ascontiguousarray(np.asarray(x, np.float32)).view(np.uint32)
    return (((u + 0x7FFF) + ((u >> 16) & 1)) >> 16).astype(np.uint16)


def _pack_units(main_cnt, cls_cnt):
    """Assign units to 16-partition groups and partition slots.

    vp/sd units go to group-relative slots {0..3, 8..11} (two runs of 4;
    vp needs 2 adjacent slots in one run), sc units to {4..7, 12..15}.
    Balances main_cnt over groups for vp/sd and cls_cnt for sc.
    Returns upart[N_UNITS], ugroup[N_UNITS], per-group (gmain, gcls).
    """
    upart = [-1] * N_UNITS
    ugroup = [-1] * N_UNITS
    gmain = [0] * 8
    gcls = [0] * 8
    # run free-lists per group: [run0_next, run1_next] relative next slot
    reg_runs = [[0, 0] for _ in range(8)]   # used counts in runs of 4
    # vp first (need adjacency), heaviest first
    vps = sorted((ui for ui in REG_UNITS if UNITS[ui][0] == "vp"),
                 key=lambda ui: -main_cnt[ui])
    sds = sorted((ui for ui in REG_UNITS if UNITS[ui][0] == "sd"),
                 key=lambda ui: -main_cnt[ui])
    for ui in vps:
        best, bestv = -1, None
        for g in range(8):
            if max(r for r in (4 - reg_runs[g][0], 4 - reg_runs[g][1])) < 2:
                continue
            if bestv is None or gmain[g] < bestv:
                best, bestv = g, gmain[g]
        assert best >= 0, "vp packing overflow"
        g = best
        r = 0 if 4 - reg_runs[g][0] >= 2 else 1
        base = 16 * g + (0 if r == 0 else 8) + reg_runs[g][r]
        reg_runs[g][r] += 2
        upart[ui] = base
        ugroup[ui] = g
        gmain[g] += int(main_cnt[ui])
    for ui in sds:
        best, bestv = -1, None
        for g in range(8):
            if reg_runs[g][0] >= 4 and reg_runs[g][1] >= 4:
                continue
            if bestv is None or gmain[g] < bestv:
                best, bestv = g, gmain[g]
        assert best >= 0, "sd packing overflow"
        g = best
        r = 0 if reg_runs[g][0] < 4 else 1
        base = 16 * g + (0 if r == 0 else 8) + reg_runs[g][r]
        reg_runs[g][r] += 1
        upart[ui] = base
        ugroup[ui] = g
        gmain[g] += int(main_cnt[ui])
    # sc units into slots {4..7, 12..15}
    sc_runs = [[0, 0] for _ in range(8)]
    scs = sorted(SC_UNITS, key=lambda ui: -cls_cnt[ui])
    for ui in scs:
        best, bestv = -1, None
        for g in range(8):
            if sc_runs[g][0] >= 4 and sc_runs[g][1] >= 4:
                continue
            if bestv is None or gcls[g] < bestv:
                best, bestv = g, gcls[g]
        assert best >= 0, "sc packing overflow"
        g = best
        r = 0 if sc_runs[g][0] < 4 else 1
        base = 16 * g + (4 if r == 0 else 12) + sc_runs[g][r]
        sc_runs[g][r] += 1
        upart[ui] = base
        ugroup[ui] = g
        gcls[g] += int(cls_cnt[ui])
    return upart, ugroup, gmain, gcls


def _first_mask(cids):
    """Boolean mask selecting one 'first' entry per distinct cell id."""
    order = np.argsort(cids, kind="stable")
    sc = cids[order]
    fs = np.ones(len(cids), np.bool_)
    if len(cids):
        fs[1:] = sc[1:] != sc[:-1]
    mask = np.zeros(len(cids), np.bool_)
    mask[order] = fs
    return mask


# ln(1+y) on [0,1], degree-5 power-basis fit (max abs err ~1e-5)
_SPC = (9.975032552178575e-06, 0.9992354838332733, -0.4902307234234048,
        0.28527268109057763, -0.13158182508877841, 0.030449004538680213)


def _build_bass(NVS, C0S, WB, NCLS):
    nc = bacc.Bacc("TRN2", target_bir_lowering=False)
    NIS = NVS // 16
    MEGA = nc.dram_tensor("mega", [128, WB], mybir.dt.uint8, kind="ExternalInput")
    OUT = nc.dram_tensor("out", [128, 12], mybir.dt.float32,
                         kind="ExternalOutput")

    o_gs = 3072                     # gather dst; NOT covered by any DMA
    o_idx = o_gs + 2 * NVS          # (avoids a WAW fence before the gather)
    o_mc = o_idx + 2 * NIS
    o_T = ((o_mc + NCLS + 3) // 4) * 4
    o_tms = o_T + 3072              # small targets right after dense T

    f32 = mybir.dt.float32
    bf16 = mybir.dt.bfloat16
    u16 = mybir.dt.uint16
    u32 = mybir.dt.uint32
    AL = mybir.AluOpType
    ACT = mybir.ActivationFunctionType
    with tile.TileContext(nc) as tc:
        with tc.tile_pool(name="p", bufs=1) as pool:
            mega = pool.tile([128, WB], mybir.dt.uint8)
            # DMA issue order = consumption order: chunk 0's inputs (data0,
            # T0) first so the scalar engine starts ASAP; then data1 + idx
            # for the gather; then chunk 1's targets.  The gather dst (gs)
            # is covered by NO DMA, avoiding a WAW fence before the gather.
            nc.sync.dma_start(mega[:, 0:3072], MEGA[:, 0:3072])
            nc.scalar.dma_start(mega[:, o_T:o_T + 1536],
                                MEGA[:, o_T:o_T + 1536])
            nc.sync.dma_start(mega[:, o_idx:o_T], MEGA[:, o_idx:o_T])
            nc.scalar.dma_start(mega[:, o_T + 1536:WB],
                                MEGA[:, o_T + 1536:WB])

            # warm activations: Exp and Square share one act table; no other
            # activation functions are used anywhere -> no mid-kernel loads
            warm = pool.tile([128, 4], f32)
            nc.scalar.activation(warm[:, 0:2], warm[:, 2:4], ACT.Exp)
            nc.scalar.activation(warm[:, 0:2], warm[:, 2:4], ACT.Square)

            data_v = mega[:, 0:3072].bitcast(bf16)               # [128,1536]
            gs_v = mega[:, o_gs:o_gs + 2 * NVS].bitcast(bf16)    # [128,NVS]
            idx_v = mega[:, o_idx:o_idx + 2 * NIS].bitcast(u16)
            mc_v = mega[:, o_mc:o_mc + NCLS]                     # u8

            P = pool.tile([128, 12], f32)

            # ---- small gather: dup-cell entries + cls logit pairs, landing
            # directly behind the dense data so dense chunk 1 covers them --
            nc.gpsimd.indirect_copy(
                gs_v, data_v, idx_v, i_know_ap_gather_is_preferred=True
            )

            # smooth-l1 partial sums for one [128, w] bf16 diff tile D via
            #   sl1(d) = 0.5*d^2 - 0.5*r^2,  r = relu(|d| - 1)
            # DVE computes r (sign-clear then clamp); ACT Square with free
            # accumulation produces both sums on the scalar engine.
            def sl1_vops(D, w):
                Ab = pool.tile([128, w], bf16)
                nc.vector.tensor_scalar(Ab[:].bitcast(u16), D[:].bitcast(u16),
                                        0x7FFF, None, AL.bitwise_and)
                R = pool.tile([128, w], bf16)
                nc.vector.tensor_scalar(R[:], Ab[:], 1.0, 0.0,
                                        AL.subtract, AL.max)
                return R

            def sl1_sop(D, R, w, pc):
                SQ = pool.tile([128, w], bf16)
                nc.scalar.activation(SQ[:], D[:], ACT.Square,
                                     accum_out=P[:, pc:pc + 1])
                SR = pool.tile([128, w], bf16)
                nc.scalar.activation(SR[:], R[:], ACT.Square,
                                     accum_out=P[:, pc + 1:pc + 2])

            # ---- dense chunk 0 (cols 0:768) ------------------------------
            D0 = pool.tile([128, 768], bf16)
            nc.vector.tensor_tensor(D0[:], data_v[:, 0:768],
                                    mega[:, o_T:o_T + 1536].bitcast(bf16),
                                    op=AL.subtract)
            R0 = sl1_vops(D0, 768)

            # ---- cls prep (gather-dependent, tiny) -----------------------
            dc = pool.tile([128, NCLS], f32)
            nc.vector.tensor_tensor(dc[:], gs_v[:, C0S:NVS:2],
                                    gs_v[:, C0S + 1:NVS:2], op=AL.subtract)
            adc = pool.tile([128, NCLS], f32)
            nc.vector.tensor_scalar(adc[:].bitcast(u32), dc[:].bitcast(u32),
                                    0x7FFFFFFF, None, AL.bitwise_and)

            # ---- dense chunk 1 (cols 768:1536 + the gathered columns) ----
            W1 = 768 + NVS
            D1 = pool.tile([128, W1], bf16)
            nc.vector.tensor_tensor(
                D1[:], mega[:, 1536:o_gs + 2 * NVS].bitcast(bf16),
                mega[:, o_T + 1536:o_tms + 2 * NVS].bitcast(bf16),
                op=AL.subtract)
            R1 = sl1_vops(D1, W1)

            # ---- scalar engine stream ------------------------------------
            sl1_sop(D0, R0, 768, 0)
            # y = exp(-|dc|), emitted before chunk 1's squares so the DVE
            # cls-poly tail overlaps them
            y = pool.tile([128, NCLS], f32)
            nc.scalar.activation(y[:], adc[:], ACT.Exp, scale=-1.0)
            sl1_sop(D1, R1, W1, 2)

            # ---- cls tail: ce = relu(dc) + ln1p(y) via degree-3 poly -----
            c0, c1, c2, c3 = (0.0005027216331514595, 0.9823971197982746,
                              -0.3971182964499659, 0.10774685617806001)
            y2 = pool.tile([128, NCLS], f32)
            nc.vector.tensor_tensor(y2[:], y[:], y[:], op=AL.mult)
            q0 = pool.tile([128, NCLS], f32)    # c0 + c1 y
            nc.vector.tensor_scalar(q0[:], y[:], c1, c0, AL.mult, AL.add)
            q1 = pool.tile([128, NCLS], f32)    # c2 + c3 y
            nc.vector.tensor_scalar(q1[:], y[:], c3, c2, AL.mult, AL.add)
            t1 = pool.tile([128, NCLS], f32)
            nc.vector.tensor_tensor(t1[:], y2[:], q1[:], op=AL.mult)
            r = pool.tile([128, NCLS], f32)
            nc.vector.tensor_scalar(r[:], dc[:], 0.0, None, AL.max)
            s1 = pool.tile([128, NCLS], f32)
            nc.vector.tensor_tensor(s1[:], q0[:], t1[:], op=AL.add)
            ce = pool.tile([128, NCLS], f32)
            nc.vector.tensor_tensor(ce[:], s1[:], r[:], op=AL.add)
            cj = pool.tile([128, NCLS], f32)
            nc.vector.tensor_tensor(cj[:], ce[:], mc_v, op=AL.mult)
            nc.vector.tensor_reduce(P[:, 4:5], cj[:],
                                    axis=mybir.AxisListType.X, op=AL.add)
            nc.vector.memset(P[:, 5:12], 0.0)

            nc.sync.dma_start(OUT[:, :], P[:])
    nc.compile()
    return nc


def kernel(**inputs):
    score = np.asarray(inputs["score"], dtype=np.float32)[0]            # [20,H,W]
    vp = np.asarray(inputs["vertical_pred"], dtype=np.float32)[0]
    side = np.asarray(inputs["side_refinement"], dtype=np.float32)[0]   # [10,H,W]
    pidx = np.asarray(inputs["positive"])
    nidx = np.asarray(inputs["negative"])
    vidx = np.asarray(inputs["vertical_reg_idx"])
    vtgt = np.asarray(inputs["vertical_reg_tgt"], dtype=np.float32)
    sidx = np.asarray(inputs["side_reg_idx"])
    stgt = np.asarray(inputs["side_reg_tgt"], dtype=np.float32)

    score_bf = _bf16(score.reshape(2 * K, HW))
    vp_bf = _bf16(vp.reshape(2 * K, HW))
    side_bf = _bf16(side.reshape(K, HW))
    vtgt_bf = _bf16(vtgt)      # [Nv,2] u16
    stgt_bf = _bf16(stgt)      # [No]   u16

    def fields(idx):
        x = idx[:, 0].astype(np.int64)
        y = idx[:, 1].astype(np.int64)
        a = idx[:, 2].astype(np.int64)
        pos = y * W + x
        return a, pos // PPC, pos % PPC

    va, vcore, vposl = fields(vidx)
    sa, score_, sposl = fields(sidx)
    pa, pcore, pposl = fields(pidx)
    na, ncore, nposl = fields(nidx)

    # --- per-entry unit / in-row-offset ----------------------------------
    v_h = vposl // COLS
    v_u = (vposl % COLS).astype(np.int64)
    v_unit = (va * 2 + v_h).astype(np.int64)                 # vp units 0..19
    s_h = sposl // COLS
    s_u = (sposl % COLS).astype(np.int64)
    s_unit = (20 + sa * 2 + s_h).astype(np.int64)            # sd units 20..39
    p_q = pposl // QCOLS
    p_u = (2 * (pposl % QCOLS)).astype(np.int64)
    p_unit = (40 + pa * 4 + p_q).astype(np.int64)            # sc units 40..79
    n_q = nposl // QCOLS
    n_u = (2 * (nposl % QCOLS)).astype(np.int64)
    n_unit = (40 + na * 4 + n_q).astype(np.int64)

    # --- dup split: one dense anchor per distinct cell --------------------
    v_cid = (vcore * K + va) * PPC + vposl                   # vp pair-cell id
    s_cid = K * HW + (score_ * K + sa) * PPC + sposl         # sd cell id
    v_first = _first_mask(v_cid)
    s_first = _first_mask(s_cid)

    main_core = np.concatenate([vcore, score_])
    main_unit = np.concatenate([v_unit, s_unit])
    main_u = np.concatenate([v_u, s_u])
    main_t0 = np.concatenate([vtgt_bf[:, 0], stgt_bf])       # bf16 bits
    main_t1 = np.concatenate([vtgt_bf[:, 1],
                              np.zeros_like(stgt_bf)])
    main_isv = np.concatenate(
        [np.ones(len(vidx), np.bool_), np.zeros(len(sidx), np.bool_)])
    main_first = np.concatenate([v_first, s_first])

    cls_core = np.concatenate([pcore, ncore])
    cls_unit = np.concatenate([p_unit, n_unit])
    cls_u = np.concatenate([p_u, n_u])
    cls_ispos = np.concatenate(
        [np.ones(len(pidx), np.bool_), np.zeros(len(nidx), np.bool_)])

    dup_cnt = np.zeros((N_CORES, N_UNITS), np.int64)
    dsel_all = ~main_first
    np.add.at(dup_cnt, (main_core[dsel_all], main_unit[dsel_all]), 1)
    cls_cnt = np.zeros((N_CORES, N_UNITS), np.int64)
    np.add.at(cls_cnt, (cls_core, cls_unit), 2)

    packs = [_pack_units(dup_cnt[c], cls_cnt[c]) for c in range(N_CORES)]
    c0s = max(max(p[2]) for p in packs)
    c0s += c0s % 2
    max_cls = max(max(p[3]) for p in packs)
    NVS = c0s + max_cls
    NVS = max(16, ((NVS + 15) // 16) * 16)
    NCLS = (NVS - c0s) // 2
    NIS = NVS // 16
    o_gs = 3072
    o_idx = o_gs + 2 * NVS
    o_mc = o_idx + 2 * NIS
    o_T = ((o_mc + NCLS + 3) // 4) * 4
    o_tms = o_T + 3072
    WB = ((o_tms + 2 * NVS + 3) // 4) * 4

    key = (NVS, c0s)
    if key not in _cache:
        _cache[key] = _build_bass(NVS, c0s, WB, NCLS)
    nc = _cache[key]

    in_maps = []
    wvec_v = np.zeros((N_CORES, 128), np.float32)
    wvec_o = np.zeros((N_CORES, 128), np.float32)
    for c in range(N_CORES):
        upart, ugroup, gmain, gcls = packs[c]
        uparta = np.asarray(upart, np.int64)

        # dense data tile (bf16 bits) --------------------------------------
        df = np.zeros((128, COLS), np.uint16)
        base = c * PPC
        for ui, (kind, a, hq) in enumerate(UNITS):
            p0 = upart[ui]
            if kind == "vp":
                sl = slice(base + hq * COLS, base + (hq + 1) * COLS)
                df[p0] = vp_bf[2 * a, sl]
                df[p0 + 1] = vp_bf[2 * a + 1, sl]
                wvec_v[c, p0] = wvec_v[c, p0 + 1] = 1.0 / (2.0 * NV_REG)
            elif kind == "sd":
                sl = slice(base + hq * COLS, base + (hq + 1) * COLS)
                df[p0] = side_bf[a, sl]
                wvec_o[c, p0] = 1.0 / NO_REG
            else:  # sc, pair-interleaved quarter
                sl = slice(base + hq * QCOLS, base + (hq + 1) * QCOLS)
                df[p0, 0::2] = score_bf[2 * a, sl]
                df[p0, 1::2] = score_bf[2 * a + 1, sl]

        # dense target tile: data copy, then first-occurrence targets ------
        Tf = df.copy()
        msel = (main_core == c) & main_first
        Tf[uparta[main_unit[msel]], main_u[msel]] = main_t0[msel]
        vsel = msel & main_isv
        Tf[uparta[main_unit[vsel]] + 1, main_u[vsel]] = main_t1[vsel]

        # small gather: dup entries + cls pairs ----------------------------
        idxs = np.zeros((128, NIS), np.uint16)
        ucol = np.zeros((8, NVS), np.int64)
        mcls = np.zeros((128, NCLS), np.uint8)

        gq_main = [0] * 8
        gq_cls = [0] * 8

        def put_idx(g, col, val):
            idxs[16 * g + col % 16, col // 16] = val
            ucol[g, col] = val

        ov_p, ov_c, ov_t = [], [], []
        dsel = (main_core == c) & ~main_first
        for u, ui, t0, t1, isv in zip(main_u[dsel], main_unit[dsel],
                                      main_t0[dsel], main_t1[dsel],
                                      main_isv[dsel]):
            g = ugroup[ui]
            col = gq_main[g]
            gq_main[g] += 1
            put_idx(g, col, u)
            p0 = upart[ui]
            ov_p.append(p0); ov_c.append(col); ov_t.append(t0)
            if isv:
                ov_p.append(p0 + 1); ov_c.append(col); ov_t.append(t1)

        csel = cls_core == c
        for u, ui, ispos in zip(cls_u[csel], cls_unit[csel],
                                cls_ispos[csel]):
            g = ugroup[ui]
            i = gq_cls[g]
            gq_cls[g] += 1
            colf = c0s + 2 * i
            if ispos:
                put_idx(g, colf, u)
                put_idx(g, colf + 1, u + 1)
            else:
                put_idx(g, colf, u + 1)
                put_idx(g, colf + 1, u)
            mcls[upart[ui], i] = 1

        # small TM (single plane, bf16): default = the gathered bits, so
        # junk columns (incl. the whole cls region) subtract to exactly 0
        tms = np.empty((128, NVS), np.uint16)
        for g in range(8):
            tms[16 * g:16 * g + 16] = df[16 * g:16 * g + 16][:, ucol[g]]
        if ov_p:
            tms[np.array(ov_p), np.array(ov_c)] = np.array(ov_t, np.uint16)

        mega = np.zeros((128, WB), np.uint8)
        mega[:, 0:3072] = df.view(np.uint8)
        mega[:, o_idx:o_idx + 2 * NIS] = idxs.view(np.uint8)
        mega[:, o_mc:o_mc + NCLS] = mcls
        mega[:, o_T:o_T + 3072] = Tf.view(np.uint8)
        mega[:, o_tms:o_tms + 2 * NVS] = tms.view(np.uint8)
        in_maps.append({"mega": mega})

    res = bass_utils.run_bass_kernel_spmd(
        nc, in_maps, core_ids=list(range(N_CORES)))

    v_loss = np.float32(0.0)
    o_loss = np.float32(0.0)
    cls_sum = np.float32(0.0)
    for c in range(N_CORES):
        P = res.results[c]["out"]      # [128, 12]
        # per-partition sl1 sum: 0.5*(Sum(d^2) - Sum(r^2)), chunks 0+1
        S = 0.5 * ((P[:, 0] + P[:, 2]) - (P[:, 1] + P[:, 3]))
        m = (wvec_v[c] != 0) | (wvec_o[c] != 0)
        S = np.where(m, S, np.float32(0))
        v_loss += np.float32(np.dot(S, wvec_v[c]))
        o_loss += np.float32(np.dot(S, wvec_o[c]))
        cls_sum += np.float32(P[:, 4].sum())
    cls_loss = np.float32(cls_sum / NS)
    loss = np.float32(cls_loss + v_loss + o_loss)
    return (np.float32(loss), np.float32(cls_loss), np.float32(v_loss),
            np.float32(o_loss))


# revision 41
# speedup vs baseline: 1.1565x; 1.1354x over previous
"""CTPN loss kernel for 8 Trainium2 NeuronCores.

v3 strategy (dense smooth-L1 pass; tiny gather only for dup cells + cls):
  * The H*W=24576 spatial positions are split into 8 contiguous slices of
    3072; core c holds the dense map data for its slice as a bf16
    [128, 1536] tile of "channel-half" rows (score rows pair-interleaved
    by quarter, as the cls gather wants them).  Within each 16-partition
    gather group, vp/sd rows sit at slots {0..3, 8..11} and score rows at
    {4..7, 12..15}, so the target tile T (needed only for vp/sd rows) can
    be DMA'd with a partition-strided pattern that loads every DMA engine
    equally.
  * T holds the per-cell regression targets scattered into the data
    layout, defaulting to a copy of the data so non-anchor cells subtract
    to exactly 0.  Smooth-L1 uses
        sl1(d) = 0.5*d^2 - 0.5*(max(|d|,1)-1)^2
    with Sum(d^2) from a DVE tensor_tensor_reduce and Sum((z-1)^2) from
    one ACT Square (bias=-1) with free accumulation.  z=max(|d|,1) comes
    from bf16 bit tricks (clear sign bit; unsigned max vs 0x3F80).
  * Cells referenced by MORE than one regression entry (random index
    collisions) can hold only one dense target; the 2nd+ entries go
    through a small InstIndirectCopy gather (a few dozen columns), along
    with the 2*128 classification logits (pair-adjacent columns,
    ce = Softplus(first - second), single-table activation).
  * Per-partition partial sums go back to the host, which applies the
    per-segment divisors (1/(2*Nv), 1/No, 1/Ns) and sums across cores
    (the all-reduce of the sharding hint).  Score-row partitions of the
    dense sums are garbage (their T rows are never shipped) and are
    simply not read by the host.
"""

import sys

sys.path.insert(0, "/opt/trn_rl_repo")

import numpy as np

import concourse.bacc as bacc
import concourse.tile as tile
from concourse import mybir
from concourse import bass_utils

# ---------------- problem constants (hardcoded per contract) ----------------
H, W, K = 128, 192, 10
HW = H * W                     # 24576
N_CORES = 8
PPC = HW // N_CORES            # 3072 positions per core
COLS = 1536                    # slot width (elements) = half of PPC
QCOLS = 768                    # quarter width (score slots are pair-interleaved)
NS = 128.0
NV_REG = 20000
NO_REG = 5000

# ---- static unit tables ----------------------------------------------------
# unit kinds: 'vp' (a, h) -> 2 partitions; 'sd' (a, h) -> 1; 'sc' (a, q) -> 1
UNITS = []
for a in range(K):
    for h in range(2):
        UNITS.append(("vp", a, h))
for a in range(K):
    for h in range(2):
        UNITS.append(("sd", a, h))
for a in range(K):
    for q in range(4):
        UNITS.append(("sc", a, q))
N_UNITS = len(UNITS)  # 80
UNIT_NPART = {"vp": 2, "sd": 1, "sc": 1}
REG_UNITS = [ui for ui, u in enumerate(UNITS) if u[0] != "sc"]
SC_UNITS = [ui for ui, u in enumerate(UNITS) if u[0] == "sc"]

_cache = {}


def _bf16(x):
    """f32 ndarray -> uint16 bf16 bits (round to nearest even)."""
    u = np.ascontiguousarray(np.asarray(x, np.float32)).view(np.uint32)
    return (((u + 0x7FFF) + ((u >> 16) & 1)) >> 16).astype(np.uint16)


def _pack_units(main_cnt, cls_cnt):
    """Assign units to 16-partition groups and partition slots.

    vp/sd units go to group-relative slots {0..3, 8..11} (two runs of 4;
    vp needs 2 adjacent slots in one run), sc units to {4..7, 12..15}.
    Balances main_cnt over groups for vp/sd and cls_cnt for sc.
    Returns upart[N_UNITS], ugroup[N_UNITS], per-group (gmain, gcls).
    """
    upart = [-1] * N_UNITS
    ugroup = [-1] * N_UNITS
    gmain = [0] * 8
    gcls = [0] * 8
    # run free-lists per group: [run0_next, run1_next] relative next slot
    reg_runs = [[0, 0] for _ in range(8)]   # used counts in runs of 4
    # vp first (need adjacency), heaviest first
    vps = sorted((ui for ui in REG_UNITS if UNITS[ui][0] == "vp"),
                 key=lambda ui: -main_cnt[ui])
    sds = sorted((ui for ui in REG_UNITS if UNITS[ui][0] == "sd"),
                 key=lambda ui: -main_cnt[ui])
    for ui in vps:
        best, bestv = -1, None
        for g in range(8):
            if max(r for r in (4 - reg_runs[g][0], 4 - reg_runs[g][1])) < 2:
                continue
            if bestv is None or gmain[g] < bestv:
                best, bestv = g, gmain[g]
        assert best >= 0, "vp packing overflow"
        g = best
        r = 0 if 4 - reg_runs[g][0] >= 2 else 1
        base = 16 * g + (0 if r == 0 else 8) + reg_runs[g][r]
        reg_runs[g][r] += 2
        upart[ui] = base
        ugroup[ui] = g
        gmain[g] += int(main_cnt[ui])
    for ui in sds:
        best, bestv = -1, None
        for g in range(8):
            if reg_runs[g][0] >= 4 and reg_runs[g][1] >= 4:
                continue
            if bestv is None or gmain[g] < bestv:
                best, bestv = g, gmain[g]
        assert best >= 0, "sd packing overflow"
        g = best
        r = 0 if reg_runs[g][0] < 4 else 1
        base = 16 * g + (0 if r == 0 else 8) + reg_runs[g][r]
        reg_runs[g][r] += 1
        upart[ui] = base
        ugroup[ui] = g
        gmain[g] += int(main_cnt[ui])
    # sc units into slots {4..7, 12..15}
    sc_runs = [[0, 0] for _ in range(8)]
    scs = sorted(SC_UNITS, key=lambda ui: -cls_cnt[ui])
    for ui in scs:
        best, bestv = -1, None
        for g in range(8):
            if sc_runs[g][0] >= 4 and sc_runs[g][1] >= 4:
                continue
            if bestv is None or gcls[g] < bestv:
                best, bestv = g, gcls[g]
        assert best >= 0, "sc packing overflow"
        g = best
        r = 0 if sc_runs[g][0] < 4 else 1
        base = 16 * g + (4 if r == 0 else 12) + sc_runs[g][r]
        sc_runs[g][r] += 1
        upart[ui] = base
        ugroup[ui] = g
        gcls[g] += int(cls_cnt[ui])
    return upart, ugroup, gmain, gcls


def _first_mask(cids):
    """Boolean mask selecting one 'first' entry per distinct cell id."""
    order = np.argsort(cids, kind="stable")
    sc = cids[order]
    fs = np.ones(len(cids), np.bool_)
    if len(cids):
        fs[1:] = sc[1:] != sc[:-1]
    mask = np.zeros(len(cids), np.bool_)
    mask[order] = fs
    return mask


# ln(1+y) on [0,1], degree-5 power-basis fit (max abs err ~1e-5)
_SPC = (9.975032552178575e-06, 0.9992354838332733, -0.4902307234234048,
        0.28527268109057763, -0.13158182508877841, 0.030449004538680213)


def _build_bass(NVS, C0S, WB, NCLS):
    nc = bacc.Bacc("TRN2", target_bir_lowering=False)
    NIS = NVS // 16
    MEGA = nc.dram_tensor("mega", [128, WB], mybir.dt.uint8, kind="ExternalInput")
    OUT = nc.dram_tensor("out", [128, 12], mybir.dt.float32,
                         kind="ExternalOutput")

    o_gs = 3072                     # gather dst; NOT covered by any DMA
    o_idx = o_gs + 2 * NVS          # (avoids a WAW fence before the gather)
    o_mc = o_idx + 2 * NIS
    o_T = ((o_mc + NCLS + 3) // 4) * 4
    o_tms = o_T + 3072              # small targets right after dense T

    f32 = mybir.dt.float32
    bf16 = mybir.dt.bfloat16
    u16 = mybir.dt.uint16
    u32 = mybir.dt.uint32
    AL = mybir.AluOpType
    ACT = mybir.ActivationFunctionType
    with tile.TileContext(nc) as tc:
        with tc.tile_pool(name="p", bufs=1) as pool:
            mega = pool.tile([128, WB], mybir.dt.uint8)
            # DMA issue order = consumption order: chunk 0's inputs (data0,
            # T0) first so the scalar engine starts ASAP; then data1 + idx
            # for the gather; then chunk 1's targets.  The gather dst (gs)
            # is covered by NO DMA, avoiding a WAW fence before the gather.
            nc.sync.dma_start(mega[:, 0:3072], MEGA[:, 0:3072])
            nc.scalar.dma_start(mega[:, o_T:o_T + 1536],
                                MEGA[:, o_T:o_T + 1536])
            nc.sync.dma_start(mega[:, o_idx:o_T], MEGA[:, o_idx:o_T])
            nc.scalar.dma_start(mega[:, o_T + 1536:WB],
                                MEGA[:, o_T + 1536:WB])

            # warm activations: Exp and Square share one act table; no other
            # activation functions are used anywhere -> no mid-kernel loads
            warm = pool.tile([128, 4], f32)
            nc.scalar.activation(warm[:, 0:2], warm[:, 2:4], ACT.Exp)
            nc.scalar.activation(warm[:, 0:2], warm[:, 2:4], ACT.Square)

            data_v = mega[:, 0:3072].bitcast(bf16)               # [128,1536]
            gs_v = mega[:, o_gs:o_gs + 2 * NVS].bitcast(bf16)    # [128,NVS]
            idx_v = mega[:, o_idx:o_idx + 2 * NIS].bitcast(u16)
            mc_v = mega[:, o_mc:o_mc + NCLS]                     # u8

            P = pool.tile([128, 12], f32)

            # ---- small gather: dup-cell entries + cls logit pairs, landing
            # directly behind the dense data so dense chunk 1 covers them --
            nc.gpsimd.indirect_copy(
                gs_v, data_v, idx_v, i_know_ap_gather_is_preferred=True
            )

            # smooth-l1 partial sums for one [128, w] bf16 diff tile D via
            #   sl1(d) = 0.5*d^2 - 0.5*r^2,  r = relu(|d| - 1)
            # DVE computes r (sign-clear then clamp); ACT Square with free
            # accumulation produces both sums on the scalar engine.
            def sl1_vops(D, w):
                Ab = pool.tile([128, w], bf16)
                nc.vector.tensor_scalar(Ab[:].bitcast(u16), D[:].bitcast(u16),
                                        0x7FFF, None, AL.bitwise_and)
                R = pool.tile([128, w], bf16)
                nc.vector.tensor_scalar(R[:], Ab[:], 1.0, 0.0,
                                        AL.subtract, AL.max)
                return R

            def sl1_sop(D, R, w, pc):
                SQ = pool.tile([128, w], bf16)
                nc.scalar.activation(SQ[:], D[:], ACT.Square,
                                     accum_out=P[:, pc:pc + 1])
                SR = pool.tile([128, w], bf16)
                nc.scalar.activation(SR[:], R[:], ACT.Square,
                                     accum_out=P[:, pc + 1:pc + 2])

            # ---- dense chunk 0 (cols 0:768) ------------------------------
            D0 = pool.tile([128, 768], bf16)
            nc.vector.tensor_tensor(D0[:], data_v[:, 0:768],
                                    mega[:, o_T:o_T + 1536].bitcast(bf16),
                                    op=AL.subtract)
            R0 = sl1_vops(D0, 768)

            # ---- cls prep (gather-dependent, tiny) -----------------------
            dc = pool.tile([128, NCLS], f32)
            nc.vector.tensor_tensor(dc[:], gs_v[:, C0S:NVS:2],
                                    gs_v[:, C0S + 1:NVS:2], op=AL.subtract)
            adc = pool.tile([128, NCLS], f32)
            nc.vector.tensor_scalar(adc[:].bitcast(u32), dc[:].bitcast(u32),
                                    0x7FFFFFFF, None, AL.bitwise_and)

            # ---- dense chunk 1 (cols 768:1536 + the gathered columns) ----
            W1 = 768 + NVS
            D1 = pool.tile([128, W1], bf16)
            nc.vector.tensor_tensor(
                D1[:], mega[:, 1536:o_gs + 2 * NVS].bitcast(bf16),
                mega[:, o_T + 1536:o_tms + 2 * NVS].bitcast(bf16),
                op=AL.subtract)
            R1 = sl1_vops(D1, W1)

            # ---- scalar engine stream ------------------------------------
            sl1_sop(D0, R0, 768, 0)
            # y = exp(-|dc|), emitted before chunk 1's squares so the DVE
            # cls-poly tail overlaps them
            y = pool.tile([128, NCLS], f32)
            nc.scalar.activation(y[:], adc[:], ACT.Exp, scale=-1.0)
            sl1_sop(D1, R1, W1, 2)

            # ---- cls tail: ce = relu(dc) + ln1p(y), deg-3 poly, with
            # scalar_tensor_tensor fusions and a fused masked-sum ----------
            c0, c1, c2, c3 = (0.0005027216331514595, 0.9823971197982746,
                              -0.3971182964499659, 0.10774685617806001)
            y2 = pool.tile([128, NCLS], f32)
            nc.vector.tensor_tensor(y2[:], y[:], y[:], op=AL.mult)
            q1 = pool.tile([128, NCLS], f32)    # c2 + c3 y
            nc.vector.tensor_scalar(q1[:], y[:], c3, c2, AL.mult, AL.add)
            t1 = pool.tile([128, NCLS], f32)
            nc.vector.tensor_tensor(t1[:], y2[:], q1[:], op=AL.mult)
            r = pool.tile([128, NCLS], f32)
            nc.vector.tensor_scalar(r[:], dc[:], 0.0, None, AL.max)
            a = pool.tile([128, NCLS], f32)     # c1*y + t1
            nc.vector.scalar_tensor_tensor(a[:], y[:], c1, t1[:],
                                           AL.mult, AL.add)
            ce = pool.tile([128, NCLS], f32)    # (a + c0) + r
            nc.vector.scalar_tensor_tensor(ce[:], a[:], c0, r[:],
                                           AL.add, AL.add)
            cj = pool.tile([128, NCLS], f32)    # ce * mask, accum -> P4
            nc.vector.scalar_tensor_tensor(cj[:], ce[:], 0.0, mc_v,
                                           AL.add, AL.mult,
                                           accum_out=P[:, 4:5])
            nc.vector.memset(P[:, 5:12], 0.0)

            nc.sync.dma_start(OUT[:, :], P[:])
    nc.compile()
    return nc


def kernel(**inputs):
    score = np.asarray(inputs["score"], dtype=np.float32)[0]            # [20,H,W]
    vp = np.asarray(inputs["vertical_pred"], dtype=np.float32)[0]
    side = np.asarray(inputs["side_refinement"], dtype=np.float32)[0]   # [10,H,W]
    pidx = np.asarray(inputs["positive"])
    nidx = np.asarray(inputs["negative"])
    vidx = np.asarray(inputs["vertical_reg_idx"])
    vtgt = np.asarray(inputs["vertical_reg_tgt"], dtype=np.float32)
    sidx = np.asarray(inputs["side_reg_idx"])
    stgt = np.asarray(inputs["side_reg_tgt"], dtype=np.float32)

    score_bf = _bf16(score.reshape(2 * K, HW))
    vp_bf = _bf16(vp.reshape(2 * K, HW))
    side_bf = _bf16(side.reshape(K, HW))
    vtgt_bf = _bf16(vtgt)      # [Nv,2] u16
    stgt_bf = _bf16(stgt)      # [No]   u16

    def fields(idx):
        x = idx[:, 0].astype(np.int64)
        y = idx[:, 1].astype(np.int64)
        a = idx[:, 2].astype(np.int64)
        pos = y * W + x
        return a, pos // PPC, pos % PPC

    va, vcore, vposl = fields(vidx)
    sa, score_, sposl = fields(sidx)
    pa, pcore, pposl = fields(pidx)
    na, ncore, nposl = fields(nidx)

    # --- per-entry unit / in-row-offset ----------------------------------
    v_h = vposl // COLS
    v_u = (vposl % COLS).astype(np.int64)
    v_unit = (va * 2 + v_h).astype(np.int64)                 # vp units 0..19
    s_h = sposl // COLS
    s_u = (sposl % COLS).astype(np.int64)
    s_unit = (20 + sa * 2 + s_h).astype(np.int64)            # sd units 20..39
    p_q = pposl // QCOLS
    p_u = (2 * (pposl % QCOLS)).astype(np.int64)
    p_unit = (40 + pa * 4 + p_q).astype(np.int64)            # sc units 40..79
    n_q = nposl // QCOLS
    n_u = (2 * (nposl % QCOLS)).astype(np.int64)
    n_unit = (40 + na * 4 + n_q).astype(np.int64)

    # --- dup split: one dense anchor per distinct cell --------------------
    v_cid = (vcore * K + va) * PPC + vposl                   # vp pair-cell id
    s_cid = K * HW + (score_ * K + sa) * PPC + sposl         # sd cell id
    v_first = _first_mask(v_cid)
    s_first = _first_mask(s_cid)

    main_core = np.concatenate([vcore, score_])
    main_unit = np.concatenate([v_unit, s_unit])
    main_u = np.concatenate([v_u, s_u])
    main_t0 = np.concatenate([vtgt_bf[:, 0], stgt_bf])       # bf16 bits
    main_t1 = np.concatenate([vtgt_bf[:, 1],
                              np.zeros_like(stgt_bf)])
    main_isv = np.concatenate(
        [np.ones(len(vidx), np.bool_), np.zeros(len(sidx), np.bool_)])
    main_first = np.concatenate([v_first, s_first])

    cls_core = np.concatenate([pcore, ncore])
    cls_unit = np.concatenate([p_unit, n_unit])
    cls_u = np.concatenate([p_u, n_u])
    cls_ispos = np.concatenate(
        [np.ones(len(pidx), np.bool_), np.zeros(len(nidx), np.bool_)])

    dup_cnt = np.zeros((N_CORES, N_UNITS), np.int64)
    dsel_all = ~main_first
    np.add.at(dup_cnt, (main_core[dsel_all], main_unit[dsel_all]), 1)
    cls_cnt = np.zeros((N_CORES, N_UNITS), np.int64)
    np.add.at(cls_cnt, (cls_core, cls_unit), 2)

    packs = [_pack_units(dup_cnt[c], cls_cnt[c]) for c in range(N_CORES)]
    c0s = max(max(p[2]) for p in packs)
    c0s += c0s % 2
    max_cls = max(max(p[3]) for p in packs)
    NVS = c0s + max_cls
    NVS = max(16, ((NVS + 15) // 16) * 16)
    NCLS = (NVS - c0s) // 2
    NIS = NVS // 16
    o_gs = 3072
    o_idx = o_gs + 2 * NVS
    o_mc = o_idx + 2 * NIS
    o_T = ((o_mc + NCLS + 3) // 4) * 4
    o_tms = o_T + 3072
    WB = ((o_tms + 2 * NVS + 3) // 4) * 4

    key = (NVS, c0s)
    if key not in _cache:
        _cache[key] = _build_bass(NVS, c0s, WB, NCLS)
    nc = _cache[key]

    in_maps = []
    wvec_v = np.zeros((N_CORES, 128), np.float32)
    wvec_o = np.zeros((N_CORES, 128), np.float32)
    for c in range(N_CORES):
        upart, ugroup, gmain, gcls = packs[c]
        uparta = np.asarray(upart, np.int64)

        # dense data tile (bf16 bits) --------------------------------------
        df = np.zeros((128, COLS), np.uint16)
        base = c * PPC
        for ui, (kind, a, hq) in enumerate(UNITS):
            p0 = upart[ui]
            if kind == "vp":
                sl = slice(base + hq * COLS, base + (hq + 1) * COLS)
                df[p0] = vp_bf[2 * a, sl]
                df[p0 + 1] = vp_bf[2 * a + 1, sl]
                wvec_v[c, p0] = wvec_v[c, p0 + 1] = 1.0 / (2.0 * NV_REG)
            elif kind == "sd":
                sl = slice(base + hq * COLS, base + (hq + 1) * COLS)
                df[p0] = side_bf[a, sl]
                wvec_o[c, p0] = 1.0 / NO_REG
            else:  # sc, pair-interleaved quarter
                sl = slice(base + hq * QCOLS, base + (hq + 1) * QCOLS)
                df[p0, 0::2] = score_bf[2 * a, sl]
                df[p0, 1::2] = score_bf[2 * a + 1, sl]

        # dense target tile: data copy, then first-occurrence targets ------
        Tf = df.copy()
        msel = (main_core == c) & main_first
        Tf[uparta[main_unit[msel]], main_u[msel]] = main_t0[msel]
        vsel = msel & main_isv
        Tf[uparta[main_unit[vsel]] + 1, main_u[vsel]] = main_t1[vsel]

        # small gather: dup entries + cls pairs ----------------------------
        idxs = np.zeros((128, NIS), np.uint16)
        ucol = np.zeros((8, NVS), np.int64)
        mcls = np.zeros((128, NCLS), np.uint8)

        gq_main = [0] * 8
        gq_cls = [0] * 8

        def put_idx(g, col, val):
            idxs[16 * g + col % 16, col // 16] = val
            ucol[g, col] = val

        ov_p, ov_c, ov_t = [], [], []
        dsel = (main_core == c) & ~main_first
        for u, ui, t0, t1, isv in zip(main_u[dsel], main_unit[dsel],
                                      main_t0[dsel], main_t1[dsel],
                                      main_isv[dsel]):
            g = ugroup[ui]
            col = gq_main[g]
            gq_main[g] += 1
            put_idx(g, col, u)
            p0 = upart[ui]
            ov_p.append(p0); ov_c.append(col); ov_t.append(t0)
            if isv:
                ov_p.append(p0 + 1); ov_c.append(col); ov_t.append(t1)

        csel = cls_core == c
        for u, ui, ispos in zip(cls_u[csel], cls_unit[csel],
                                cls_ispos[csel]):
            g = ugroup[ui]
            i = gq_cls[g]
            gq_cls[g] += 1
            colf = c0s + 2 * i
            if ispos:
                put_idx(g, colf, u)
                put_idx(g, colf + 1, u + 1)
            else:
                put_idx(g, colf, u + 1)
                put_idx(g, colf + 1, u)
            mcls[upart[ui], i] = 1

        # small TM (single plane, bf16): default = the gathered bits, so
        # junk columns (incl. the whole cls region) subtract to exactly 0
        tms = np.empty((128, NVS), np.uint16)
        for g in range(8):
            tms[16 * g:16 * g + 16] = df[16 * g:16 * g + 16][:, ucol[g]]
        if ov_p:
            tms[np.array(ov_p), np.array(ov_c)] = np.array(ov_t, np.uint16)

        mega = np.zeros((128, WB), np.uint8)
        mega[:, 0:3072] = df.view(np.uint8)
        mega[:, o_idx:o_idx + 2 * NIS] = idxs.view(np.uint8)
        mega[:, o_mc:o_mc + NCLS] = mcls
        mega[:, o_T:o_T + 3072] = Tf.view(np.uint8)
        mega[:, o_tms:o_tms + 2 * NVS] = tms.view(np.uint8)
        in_maps.append({"mega": mega})

    res = bass_utils.run_bass_kernel_spmd(
        nc, in_maps, core_ids=list(range(N_CORES)))

    v_loss = np.float32(0.0)
    o_loss = np.float32(0.0)
    cls_sum = np.float32(0.0)
    for c in range(N_CORES):
        P = res.results[c]["out"]      # [128, 12]
        # per-partition sl1 sum: 0.5*(Sum(d^2) - Sum(r^2)), chunks 0+1
        S = 0.5 * ((P[:, 0] + P[:, 2]) - (P[:, 1] + P[:, 3]))
        m = (wvec_v[c] != 0) | (wvec_o[c] != 0)
        S = np.where(m, S, np.float32(0))
        v_loss += np.float32(np.dot(S, wvec_v[c]))
        o_loss += np.float32(np.dot(S, wvec_o[c]))
        cls_sum += np.float32(P[:, 4].sum())
    cls_loss = np.float32(cls_sum / NS)
    loss = np.float32(cls_loss + v_loss + o_loss)
    return (np.float32(loss), np.float32(cls_loss), np.float32(v_loss),
            np.float32(o_loss))
